# revision 4
# baseline (speedup 1.0000x reference)
"""Trainium2 Bass kernel for a 3-layer GraphSAGE GNN (CellTypeGNN).

Strategy (graph/data parallel over 8 NeuronCores):
- Nodes are sharded by range: core c owns nodes [c*6250, (c+1)*6250).
- Edges are assigned to the core owning their destination node, grouped into
  128-node destination windows, and packed into 128-edge subtiles.
- Messages x[src] are fetched with batched SWDGE dma_gather (fp16, 256B rows)
  from a full replica of x in DRAM. int16 gather indices only reach 32767, so
  x is split into lo (nodes < 25000) and hi tables; edges are segregated by
  source half within each window.
- Segment-mean aggregation: for each 128-edge subtile an fp16 one-hot matrix
  scaled by 1/deg(dst) is built on the vector engine with a single
  tensor_scalar(is_equal, mult); PE matmuls msg^T @ onehot accumulate the mean
  into PSUM per destination window, feature-major.
- SAGE linear: fp16 weight matmuls (Wl on aggregated mean + Wr on x) into the
  same PSUM bank; LayerNorm stats computed node-major (PE transpose),
  affine+GELU fused into one scalar-engine activation, residual on DVE.
- After layers 1 and 2 the updated fp16 node features are AllGathered across
  the 8 cores so the next layer can gather arbitrary source nodes.
- Classifier LayerNorm affine is folded into the final linear on the host.
"""

import numpy as np
from contextlib import ExitStack

import concourse.bass as bass
import concourse.tile as tile
from concourse import bacc, mybir
from concourse.bass_utils import run_bass_kernel_spmd

P = 128
N, E, D = 50000, 800000, 128
DOUT, NCLS = 64, 40
NCORES = 8
NPC = N // NCORES            # 6250 nodes per core
W = (NPC + P - 1) // P       # 49 windows per core
NPAD = W * P                 # 6272 padded per-core node count
HALF = 25000                 # lo/hi table split (int16 gather indices)
G = 32                       # gather chunk size in 128-edge subtiles
LN_EPS = 1e-5

f32 = mybir.dt.float32
f16 = mybir.dt.float16
i16 = mybir.dt.int16

_cache = {}


def _schedule(edge_index):
    """Host-side edge preprocessing. Returns per-core gather/one-hot arrays
    plus the SPMD-uniform window subtile schedule."""
    src = edge_index[0].astype(np.int64)
    dst = edge_index[1].astype(np.int64)
    deg = np.bincount(dst, minlength=N)
    invdeg_all = (1.0 / np.maximum(deg, 1)).astype(np.float32)

    core = dst // NPC
    loc = dst - core * NPC
    win = loc >> 7
    rel = (loc & 127).astype(np.float32)
    half = (src >= HALF).astype(np.int64)

    # group id: (core, window, half); edges sorted by group
    gid = (core * W + win) * 2 + half
    order = np.argsort(gid, kind="stable")
    gid_s = gid[order]
    counts = np.bincount(gid_s, minlength=NCORES * W * 2).reshape(NCORES, W, 2)

    # SPMD-uniform subtile counts per (window, half)
    ntiles = (counts.max(axis=0) + P - 1) // P  # [W, 2]
    empty = ntiles.sum(axis=1) == 0
    ntiles[empty, 0] = 1
    TL = int(ntiles[:, 0].sum())
    TH = int(ntiles[:, 1].sum())
    startA = np.concatenate([[0], np.cumsum(ntiles[:, 0])[:-1]]).astype(np.int64)
    startB = np.concatenate([[0], np.cumsum(ntiles[:, 1])[:-1]]).astype(np.int64)

    # per-core stream arrays
    idxA = np.zeros((NCORES, P, TL), np.int16)
    idxB = np.zeros((NCORES, P, TH), np.int16)
    relA = np.full((NCORES, P, TL), -1.0, np.float32)
    relB = np.full((NCORES, P, TH), -1.0, np.float32)
    invA = np.zeros((NCORES, P, TL), np.float32)
    invB = np.zeros((NCORES, P, TH), np.float32)

    # vectorized placement: rank of each edge within its (core, win, half) group
    grp_start_per_edge = np.concatenate([[0], np.cumsum(np.bincount(
        gid_s, minlength=NCORES * W * 2))])[gid_s]
    rank = np.arange(len(gid_s)) - grp_start_per_edge
    e_core = core[order]
    e_win = win[order]
    e_half = half[order]
    e_src = src[order]
    e_rel = rel[order]
    e_inv = invdeg_all[dst[order]]
    pos = np.where(e_half == 0, startA[e_win], startB[e_win]) + (rank >> 7)
    prt = rank & 127

    mA = e_half == 0
    idxA[e_core[mA], prt[mA], pos[mA]] = e_src[mA].astype(np.int16)
    relA[e_core[mA], prt[mA], pos[mA]] = e_rel[mA]
    invA[e_core[mA], prt[mA], pos[mA]] = e_inv[mA]
    mB = ~mA
    idxB[e_core[mB], prt[mB], pos[mB]] = (e_src[mB] - HALF).astype(np.int16)
    relB[e_core[mB], prt[mB], pos[mB]] = e_rel[mB]
    invB[e_core[mB], prt[mB], pos[mB]] = e_inv[mB]

    def wrap(idx_pt):  # [P, T] slot-major -> wrapped [128, T*8] per chunk
        Tn = idx_pt.shape[1]
        out = np.zeros((128, Tn * 8), np.int16)
        for c0 in range(0, Tn, G):
            c1 = min(c0 + G, Tn)
            flat = idx_pt[:, c0:c1].T.reshape(-1)  # i = t_local*128 + p
            w16 = flat.reshape(-1, 16).T  # [16, n/16]: i -> [i%16, i//16]
            out[:, c0 * 8 : c0 * 8 + w16.shape[1]] = np.tile(w16, (8, 1))
        return out

    idxAw = np.stack([wrap(idxA[c]) for c in range(NCORES)])
    idxBw = np.stack([wrap(idxB[c]) for c in range(NCORES)])
    return dict(
        ntiles=ntiles, TL=TL, TH=TH, startA=startA, startB=startB,
        idxA=idxAw, idxB=idxBw, relA=relA, relB=relB, invA=invA, invB=invB,
    )


def _build(sched):
    """Build and compile the SPMD Bass program."""
    ntiles, TL, TH = sched["ntiles"], sched["TL"], sched["TH"]
    startA, startB = sched["startA"], sched["startB"]

    nc = bacc.Bacc("TRN2", target_bir_lowering=False, debug=False,
                   num_devices=NCORES)

    def din(name, shape, dt):
        return nc.dram_tensor(name, shape, dt, kind="ExternalInput")

    xf16_d = din("xf16", [N, D], f16)
    xfm32_d = din("xfm32", [P, NPAD], f32)
    xfm16_d = din("xfm16", [P, NPAD], f16)
    idxA_d = din("idxA", [P, TL * 8], i16)
    idxB_d = din("idxB", [P, TH * 8], i16)
    relA_d = din("relA", [P, TL], f32)
    relB_d = din("relB", [P, TH], f32)
    invA_d = din("invA", [P, TL], f32)
    invB_d = din("invB", [P, TH], f32)
    iota_d = din("iota", [P, P], f16)
    ident_d = din("ident", [P, P], f32)
    wl_d = [din(f"wl{l}", [D, D if l < 2 else DOUT], f16) for l in range(3)]
    wr_d = [din(f"wr{l}", [D, D if l < 2 else DOUT], f16) for l in range(3)]
    bl_d = [din(f"bl{l}", [D if l < 2 else DOUT, 1], f32) for l in range(3)]
    g_d = [din(f"g{l}", [D, 1], f32) for l in range(2)]
    b_d = [din(f"b{l}", [D, 1], f32) for l in range(2)]
    eps_d = din("eps", [P, 1], f32)
    wc_d = din("wc", [DOUT, NCLS], f16)
    bc_d = din("bc", [NCLS, 1], f32)
    out_d = nc.dram_tensor("out", [NCLS, NPAD], f32, kind="ExternalOutput")

    xg_own = [nc.dram_tensor(f"xg{l}_own", [NPC, D], f16) for l in range(2)]
    xg_full = [
        nc.dram_tensor(f"xg{l}_full", [N, D], f16, addr_space="Shared")
        for l in range(2)
    ]

    with tile.TileContext(nc) as tc, ExitStack() as ctx:
        cpool = ctx.enter_context(tc.tile_pool(name="const", bufs=1))
        xpool = ctx.enter_context(tc.tile_pool(name="x", bufs=1))
        idxpool = ctx.enter_context(tc.tile_pool(name="idx", bufs=3))
        msgApool = ctx.enter_context(tc.tile_pool(name="msgA", bufs=2))
        msgBpool = ctx.enter_context(tc.tile_pool(name="msgB", bufs=2))
        ohpool = ctx.enter_context(tc.tile_pool(name="oh", bufs=6))
        wkpool = ctx.enter_context(tc.tile_pool(name="wk", bufs=4))
        stpool = ctx.enter_context(tc.tile_pool(name="st", bufs=8))
        psA = ctx.enter_context(tc.tile_pool(name="psA", bufs=2, space="PSUM"))
        psC = ctx.enter_context(tc.tile_pool(name="psC", bufs=2, space="PSUM"))
        psT = ctx.enter_context(tc.tile_pool(name="psT", bufs=3, space="PSUM"))

        def load(dram, shape, dt):
            t = cpool.tile(shape, dt, name=f"c_{dram.name}")
            nc.sync.dma_start(out=t[:], in_=dram.ap())
            return t

        iota_t = load(iota_d, [P, P], f16)
        ident_t = load(ident_d, [P, P], f32)
        relA_t = load(relA_d, [P, TL], f32)
        relB_t = load(relB_d, [P, TH], f32)
        invA_t = load(invA_d, [P, TL], f32)
        invB_t = load(invB_d, [P, TH], f32)
        wl_t = [load(wl_d[l], [D, D if l < 2 else DOUT], f16) for l in range(3)]
        wr_t = [load(wr_d[l], [D, D if l < 2 else DOUT], f16) for l in range(3)]
        bl_t = [load(bl_d[l], [D if l < 2 else DOUT, 1], f32) for l in range(3)]
        g_t = [load(g_d[l], [D, 1], f32) for l in range(2)]
        b_t = [load(b_d[l], [D, 1], f32) for l in range(2)]
        eps_t = load(eps_d, [P, 1], f32)
        wc_t = load(wc_d, [DOUT, NCLS], f16)
        bc_t = load(bc_d, [NCLS, 1], f32)

        xfm32 = [xpool.tile([P, NPAD], f32, tag=f"xfm32_{i}", name=f"xfm32_{i}")
                 for i in range(2)]
        xfm16 = [xpool.tile([P, NPAD], f16, tag=f"xfm16_{i}", name=f"xfm16_{i}")
                 for i in range(2)]
        nc.sync.dma_start(out=xfm32[0][:], in_=xfm32_d.ap())
        nc.sync.dma_start(out=xfm16[0][:], in_=xfm16_d.ap())
        x3fm = xpool.tile([DOUT, NPAD], f32, tag="x3fm")
        normfm = xpool.tile([DOUT, NPAD], f16, tag="normfm")

        nchunkA = (TL + G - 1) // G
        nchunkB = (TH + G - 1) // G

        for l in range(3):
            dout = D if l < 2 else DOUT
            cur32, cur16 = xfm32[l % 2], xfm16[l % 2]
            nxt32, nxt16 = xfm32[(l + 1) % 2], xfm16[(l + 1) % 2]
            if l == 0:
                src_lo = xf16_d.ap()[:HALF, :]
                src_hi = xf16_d.ap()[HALF:, :]
            else:
                src_lo = xg_full[l - 1].ap()[:HALF, :]
                src_hi = xg_full[l - 1].ap()[HALF:, :]

            # emit gather chunks lazily; Tile pool backpressure pipelines them
            msgs = {"A": {}, "B": {}}
            issued = {"A": -1, "B": -1}

            def emit_chunk(stream, ci, l=l, src_lo=src_lo, src_hi=src_hi,
                           msgs=msgs):
                Tn = TL if stream == "A" else TH
                idxd = idxA_d if stream == "A" else idxB_d
                mpool = msgApool if stream == "A" else msgBpool
                src = src_lo if stream == "A" else src_hi
                c0 = ci * G
                cn = min(G, Tn - c0)
                nidx = cn * P
                it = idxpool.tile([P, G * 8], i16, tag="idx")
                nc.sync.dma_start(
                    out=it[:, : cn * 8], in_=idxd.ap()[:, c0 * 8 : c0 * 8 + cn * 8]
                )
                mt = mpool.tile([P, G * P], f16, tag=f"msg{stream}")
                nc.gpsimd.dma_gather(
                    mt[:, : cn * P].rearrange("p (t d) -> p t d", d=P),
                    src,
                    it[:, : cn * 8],
                    nidx,
                    nidx,
                    P,
                    single_packet=False,
                )
                msgs[stream][ci] = mt

            for w in range(W):
                nA, nB = int(ntiles[w, 0]), int(ntiles[w, 1])
                subs = [("A", int(startA[w]) + i) for i in range(nA)] + [
                    ("B", int(startB[w]) + i) for i in range(nB)
                ]
                for stream, pos in subs:
                    while issued[stream] < pos // G:
                        issued[stream] += 1
                        emit_chunk(stream, issued[stream])

                ps = psA.tile([P, P], f32, space="PSUM", tag="agg")
                for si, (stream, pos) in enumerate(subs):
                    rel_t = relA_t if stream == "A" else relB_t
                    inv_t = invA_t if stream == "A" else invB_t
                    mt = msgs[stream][pos // G]
                    t = pos % G
                    oh = ohpool.tile([P, P], f16, tag="oh")
                    nc.vector.tensor_scalar(
                        out=oh[:],
                        in0=iota_t[:],
                        scalar1=rel_t[:, pos : pos + 1],
                        scalar2=inv_t[:, pos : pos + 1],
                        op0=mybir.AluOpType.is_equal,
                        op1=mybir.AluOpType.mult,
                    )
                    nc.tensor.matmul(
                        out=ps[:],
                        lhsT=mt[:, t * P : (t + 1) * P],
                        rhs=oh[:],
                        start=(si == 0),
                        stop=(si == len(subs) - 1),
                    )
                agg16 = wkpool.tile([P, P], f16, tag="agg16")
                nc.scalar.copy(agg16[:], ps[:])

                hps = psC.tile([dout, P], f32, space="PSUM", tag="h")
                nc.tensor.matmul(out=hps[:], lhsT=wl_t[l][:], rhs=agg16[:],
                                 start=True, stop=False)
                nc.tensor.matmul(out=hps[:], lhsT=wr_t[l][:],
                                 rhs=cur16[:, w * P : (w + 1) * P],
                                 start=False, stop=True)

                cols = slice(w * P, (w + 1) * P)
                if l < 2:
                    hfm = wkpool.tile([P, P], f32, tag="hfm")
                    nc.scalar.activation(hfm[:], hps[:],
                                         mybir.ActivationFunctionType.Identity,
                                         bias=bl_t[l][:, :1])
                    tp1 = psT.tile([P, P], f32, space="PSUM", tag="tp")
                    nc.tensor.transpose(tp1[:], hfm[:], ident_t[:])
                    hnm = wkpool.tile([P, P], f32, tag="hnm")
                    nc.scalar.copy(hnm[:], tp1[:])
                    s_ = stpool.tile([P, 1], f32, tag="sum")
                    nc.vector.reduce_sum(s_[:], hnm[:], axis=mybir.AxisListType.X)
                    nmu = stpool.tile([P, 1], f32, tag="nmu")
                    nc.scalar.mul(nmu[:], s_[:], -1.0 / D)
                    xc = wkpool.tile([P, P], f32, tag="xc")
                    nc.scalar.activation(xc[:], hnm[:],
                                         mybir.ActivationFunctionType.Identity,
                                         bias=nmu[:, :1])
                    sq = wkpool.tile([P, P], f32, tag="sq")
                    ss = stpool.tile([P, 1], f32, tag="ss")
                    nc.scalar.activation(sq[:], xc[:],
                                         mybir.ActivationFunctionType.Square,
                                         accum_out=ss[:, :1])
                    sd = stpool.tile([P, 1], f32, tag="sd")
                    nc.scalar.activation(sd[:], ss[:],
                                         mybir.ActivationFunctionType.Sqrt,
                                         scale=1.0 / D, bias=eps_t[:, :1])
                    rs = stpool.tile([P, 1], f32, tag="rs")
                    nc.vector.reciprocal(rs[:], sd[:])
                    nrm = wkpool.tile([P, P], f32, tag="nrm")
                    nc.vector.tensor_scalar_mul(nrm[:], xc[:], rs[:, :1])
                    tp2 = psT.tile([P, P], f32, space="PSUM", tag="tp")
                    nc.tensor.transpose(tp2[:], nrm[:], ident_t[:])
                    gel = wkpool.tile([P, P], f32, tag="gel")
                    nc.scalar.activation(gel[:], tp2[:],
                                         mybir.ActivationFunctionType.Gelu,
                                         bias=b_t[l][:, :1], scale=g_t[l][:, :1])
                    nc.vector.tensor_add(nxt32[:, cols], gel[:], cur32[:, cols])
                    nc.any.tensor_copy(nxt16[:, cols], nxt32[:, cols])
                    tp3 = psT.tile([P, P], f32, space="PSUM", tag="tp")
                    nc.tensor.transpose(tp3[:], nxt32[:, cols], ident_t[:])
                    xnm = wkpool.tile([P, P], f16, tag="xnm")
                    nc.scalar.copy(xnm[:], tp3[:])
                    rows = min(P, NPC - w * P)
                    nc.sync.dma_start(
                        out=xg_own[l].ap()[w * P : w * P + rows, :],
                        in_=xnm[:rows, :],
                    )
                else:
                    nc.scalar.activation(x3fm[:, cols], hps[:],
                                         mybir.ActivationFunctionType.Gelu,
                                         bias=bl_t[l][:, :1])

            if l < 2:
                nc.gpsimd.collective_compute(
                    "AllGather",
                    mybir.AluOpType.bypass,
                    replica_groups=[list(range(NCORES))],
                    ins=[xg_own[l].ap()],
                    outs=[xg_full[l].ap()],
                )

        # classifier: LN (affine folded into wc) then linear
        for w in range(W):
            cols = slice(w * P, (w + 1) * P)
            tp1 = psT.tile([P, DOUT], f32, space="PSUM", tag="tp")
            nc.tensor.transpose(tp1[:], x3fm[:, cols], ident_t[:DOUT, :DOUT])
            ynm = wkpool.tile([P, DOUT], f32, tag="ynm")
            nc.scalar.copy(ynm[:], tp1[:])
            s_ = stpool.tile([P, 1], f32, tag="sum")
            nc.vector.reduce_sum(s_[:], ynm[:], axis=mybir.AxisListType.X)
            nmu = stpool.tile([P, 1], f32, tag="nmu")
            nc.scalar.mul(nmu[:], s_[:], -1.0 / DOUT)
            xc = wkpool.tile([P, DOUT], f32, tag="xc")
            nc.scalar.activation(xc[:], ynm[:],
                                 mybir.ActivationFunctionType.Identity,
                                 bias=nmu[:, :1])
            sq = wkpool.tile([P, DOUT], f32, tag="sq")
            ss = stpool.tile([P, 1], f32, tag="ss")
            nc.scalar.activation(sq[:], xc[:],
                                 mybir.ActivationFunctionType.Square,
                                 accum_out=ss[:, :1])
            sd = stpool.tile([P, 1], f32, tag="sd")
            nc.scalar.activation(sd[:], ss[:],
                                 mybir.ActivationFunctionType.Sqrt,
                                 scale=1.0 / DOUT, bias=eps_t[:, :1])
            rs = stpool.tile([P, 1], f32, tag="rs")
            nc.vector.reciprocal(rs[:], sd[:])
            nrm = wkpool.tile([P, DOUT], f32, tag="nrm")
            nc.vector.tensor_scalar_mul(nrm[:], xc[:], rs[:, :1])
            tp2 = psT.tile([DOUT, P], f32, space="PSUM", tag="tp")
            nc.tensor.transpose(tp2[:], nrm[:], ident_t[:])
            nc.scalar.copy(normfm[:, cols], tp2[:])

        NCHUNK = 512
        for c0 in range(0, NPAD, NCHUNK):
            cn = min(NCHUNK, NPAD - c0)
            ops = psC.tile([NCLS, NCHUNK], f32, space="PSUM", tag="h")
            nc.tensor.matmul(out=ops[:, :cn], lhsT=wc_t[:],
                             rhs=normfm[:, c0 : c0 + cn], start=True, stop=True)
            osb = wkpool.tile([NCLS, NCHUNK], f32, tag="osb")
            nc.scalar.activation(osb[:, :cn], ops[:, :cn],
                                 mybir.ActivationFunctionType.Identity,
                                 bias=bc_t[:, :1])
            nc.sync.dma_start(out=out_d.ap()[:, c0 : c0 + cn], in_=osb[:, :cn])

    nc.compile()
    return nc


def _prep_inputs(x, sched, weights):
    """Build per-core input maps."""
    xf16 = x.astype(np.float16)
    iota = np.broadcast_to(np.arange(P, dtype=np.float16), (P, P)).copy()
    ident = np.eye(P, dtype=np.float32)
    (Wl1, bl1, Wr1, g1, b1, Wl2, bl2, Wr2, g2, b2,
     Wl3, bl3, Wr3, gc, bc, Wc, bcls) = weights
    wcp = (gc[:, None].astype(np.float32) * Wc.astype(np.float32))
    bcp = bc.astype(np.float32) @ Wc.astype(np.float32) + bcls.astype(np.float32)
    common = {
        "xf16": xf16,
        "iota": iota,
        "ident": ident,
        "eps": np.full((P, 1), LN_EPS, np.float32),
        "wl0": Wl1.astype(np.float16), "wr0": Wr1.astype(np.float16),
        "wl1": Wl2.astype(np.float16), "wr1": Wr2.astype(np.float16),
        "wl2": Wl3.astype(np.float16), "wr2": Wr3.astype(np.float16),
        "bl0": bl1.reshape(-1, 1).astype(np.float32),
        "bl1": bl2.reshape(-1, 1).astype(np.float32),
        "bl2": bl3.reshape(-1, 1).astype(np.float32),
        "g0": g1.reshape(-1, 1).astype(np.float32),
        "b0": b1.reshape(-1, 1).astype(np.float32),
        "g1": g2.reshape(-1, 1).astype(np.float32),
        "b1": b2.reshape(-1, 1).astype(np.float32),
        "wc": wcp.astype(np.float16),
        "bc": bcp.reshape(-1, 1).astype(np.float32),
    }
    in_maps = []
    for c in range(NCORES):
        xc_ = x[c * NPC : (c + 1) * NPC].astype(np.float32)
        xfm = np.zeros((P, NPAD), np.float32)
        xfm[:, :NPC] = xc_.T
        m = dict(common)
        m.update(
            xfm32=xfm,
            xfm16=xfm.astype(np.float16),
            idxA=sched["idxA"][c],
            idxB=sched["idxB"][c],
            relA=sched["relA"][c],
            relB=sched["relB"][c],
            invA=sched["invA"][c],
            invB=sched["invB"][c],
        )
        in_maps.append(m)
    return in_maps


def kernel(x, edge_index, Wl1, bl1, Wr1, g1, b1, Wl2, bl2, Wr2, g2, b2,
           Wl3, bl3, Wr3, gc, bc, Wc, bcls):
    x = np.asarray(x)
    edge_index = np.asarray(edge_index)
    sched = _schedule(edge_index)
    key = (sched["TL"], sched["TH"], tuple(sched["ntiles"].ravel().tolist()))
    if key not in _cache:
        _cache[key] = _build(sched)
    nc = _cache[key]
    weights = (Wl1, bl1, Wr1, g1, b1, Wl2, bl2, Wr2, g2, b2,
               Wl3, bl3, Wr3, gc, bc, Wc, bcls)
    in_maps = _prep_inputs(x, sched, [np.asarray(w) for w in weights])
    res = run_bass_kernel_spmd(nc, in_maps, core_ids=list(range(NCORES)))
    out = np.empty((N, NCLS), np.float32)
    for c in range(NCORES):
        out[c * NPC : (c + 1) * NPC] = res.results[c]["out"][:, :NPC].T
    return out


# revision 6
# speedup vs baseline: 127.0597x; 127.0597x over previous
"""Trainium2 Bass kernel for a 3-layer GraphSAGE GNN (CellTypeGNN).

Strategy (graph/data parallel over 8 NeuronCores):
- Nodes are sharded by range: core c owns nodes [c*6250, (c+1)*6250).
- Edges are assigned to the core owning their destination node, grouped into
  128-node destination windows, and packed into 128-edge subtiles.
- Messages x[src] are fetched with batched SWDGE dma_gather (fp16, 256B rows)
  from a full replica of x in DRAM. int16 gather indices only reach 32767, so
  x is split into lo (nodes < 25000) and hi tables; edges are segregated by
  source half within each window.
- Segment-mean aggregation: for each 128-edge subtile an fp16 one-hot matrix
  scaled by 1/deg(dst) is built on the vector engine with a single
  tensor_scalar(is_equal, mult); PE matmuls msg^T @ onehot accumulate the mean
  into PSUM per destination window, feature-major.
- SAGE linear: fp16 weight matmuls (Wl on aggregated mean + Wr on x) into the
  same PSUM bank; LayerNorm stats computed node-major (PE transpose),
  affine+GELU fused into one scalar-engine activation, residual on DVE.
- After layers 1 and 2 the updated fp16 node features are AllGathered across
  the 8 cores so the next layer can gather arbitrary source nodes.
- Classifier LayerNorm affine is folded into the final linear on the host.
"""

import numpy as np
from contextlib import ExitStack

import concourse.bass as bass
import concourse.tile as tile
from concourse import bacc, mybir
from concourse.bass_utils import run_bass_kernel_spmd

P = 128
N, E, D = 50000, 800000, 128
DOUT, NCLS = 64, 40
NCORES = 8
NPC = N // NCORES            # 6250 nodes per core
W = (NPC + P - 1) // P       # 49 windows per core
NPAD = W * P                 # 6272 padded per-core node count
HALF = 25000                 # lo/hi table split (int16 gather indices)
G = 32                       # gather chunk size in 128-edge subtiles
LN_EPS = 1e-5

f32 = mybir.dt.float32
f16 = mybir.dt.float16
i16 = mybir.dt.int16

_cache = {}


def _schedule(edge_index):
    """Host-side edge preprocessing. Returns per-core gather/one-hot arrays
    plus the SPMD-uniform window subtile schedule."""
    src = edge_index[0].astype(np.int64)
    dst = edge_index[1].astype(np.int64)
    deg = np.bincount(dst, minlength=N)
    invdeg_all = (1.0 / np.maximum(deg, 1)).astype(np.float32)

    core = dst // NPC
    loc = dst - core * NPC
    win = loc >> 7
    rel = (loc & 127).astype(np.float32)
    half = (src >= HALF).astype(np.int64)

    # group id: (core, window, half); edges sorted by group
    gid = (core * W + win) * 2 + half
    order = np.argsort(gid, kind="stable")
    gid_s = gid[order]
    counts = np.bincount(gid_s, minlength=NCORES * W * 2).reshape(NCORES, W, 2)

    # SPMD-uniform subtile counts per (window, half)
    ntiles = (counts.max(axis=0) + P - 1) // P  # [W, 2]
    empty = ntiles.sum(axis=1) == 0
    ntiles[empty, 0] = 1
    TL = int(ntiles[:, 0].sum())
    TH = int(ntiles[:, 1].sum())
    startA = np.concatenate([[0], np.cumsum(ntiles[:, 0])[:-1]]).astype(np.int64)
    startB = np.concatenate([[0], np.cumsum(ntiles[:, 1])[:-1]]).astype(np.int64)

    # per-core stream arrays
    idxA = np.zeros((NCORES, P, TL), np.int16)
    idxB = np.zeros((NCORES, P, TH), np.int16)
    relA = np.full((NCORES, P, TL), -1.0, np.float32)
    relB = np.full((NCORES, P, TH), -1.0, np.float32)
    invA = np.zeros((NCORES, P, TL), np.float32)
    invB = np.zeros((NCORES, P, TH), np.float32)

    # vectorized placement: rank of each edge within its (core, win, half) group
    grp_start_per_edge = np.concatenate([[0], np.cumsum(np.bincount(
        gid_s, minlength=NCORES * W * 2))])[gid_s]
    rank = np.arange(len(gid_s)) - grp_start_per_edge
    e_core = core[order]
    e_win = win[order]
    e_half = half[order]
    e_src = src[order]
    e_rel = rel[order]
    e_inv = invdeg_all[dst[order]]
    pos = np.where(e_half == 0, startA[e_win], startB[e_win]) + (rank >> 7)
    prt = rank & 127

    mA = e_half == 0
    idxA[e_core[mA], prt[mA], pos[mA]] = e_src[mA].astype(np.int16)
    relA[e_core[mA], prt[mA], pos[mA]] = e_rel[mA]
    invA[e_core[mA], prt[mA], pos[mA]] = e_inv[mA]
    mB = ~mA
    idxB[e_core[mB], prt[mB], pos[mB]] = (e_src[mB] - HALF).astype(np.int16)
    relB[e_core[mB], prt[mB], pos[mB]] = e_rel[mB]
    invB[e_core[mB], prt[mB], pos[mB]] = e_inv[mB]

    def wrap(idx_pt):  # [P, T] slot-major -> wrapped [128, T*8] per chunk
        Tn = idx_pt.shape[1]
        out = np.zeros((128, Tn * 8), np.int16)
        for c0 in range(0, Tn, G):
            c1 = min(c0 + G, Tn)
            flat = idx_pt[:, c0:c1].T.reshape(-1)  # i = t_local*128 + p
            w16 = flat.reshape(-1, 16).T  # [16, n/16]: i -> [i%16, i//16]
            out[:, c0 * 8 : c0 * 8 + w16.shape[1]] = np.tile(w16, (8, 1))
        return out

    idxAw = np.stack([wrap(idxA[c]) for c in range(NCORES)])
    idxBw = np.stack([wrap(idxB[c]) for c in range(NCORES)])
    return dict(
        ntiles=ntiles, TL=TL, TH=TH, startA=startA, startB=startB,
        idxA=idxAw, idxB=idxBw, relA=relA, relB=relB, invA=invA, invB=invB,
    )


def _build(sched):
    """Build and compile the SPMD Bass program."""
    ntiles, TL, TH = sched["ntiles"], sched["TL"], sched["TH"]
    startA, startB = sched["startA"], sched["startB"]

    nc = bacc.Bacc("TRN2", target_bir_lowering=False, debug=False,
                   num_devices=NCORES)

    def din(name, shape, dt):
        return nc.dram_tensor(name, shape, dt, kind="ExternalInput")

    xf16_d = din("xf16", [N, D], f16)
    xfm32_d = din("xfm32", [P, NPAD], f32)
    xfm16_d = din("xfm16", [P, NPAD], f16)
    idxA_d = din("idxA", [P, TL * 8], i16)
    idxB_d = din("idxB", [P, TH * 8], i16)
    relA_d = din("relA", [P, TL], f32)
    relB_d = din("relB", [P, TH], f32)
    invA_d = din("invA", [P, TL], f32)
    invB_d = din("invB", [P, TH], f32)
    iota_d = din("iota", [P, P], f16)
    ident_d = din("ident", [P, P], f32)
    wl_d = [din(f"wl{l}", [D, D if l < 2 else DOUT], f16) for l in range(3)]
    wr_d = [din(f"wr{l}", [D, D if l < 2 else DOUT], f16) for l in range(3)]
    bl_d = [din(f"bl{l}", [D if l < 2 else DOUT, 1], f32) for l in range(3)]
    g_d = [din(f"g{l}", [D, 1], f32) for l in range(2)]
    b_d = [din(f"b{l}", [D, 1], f32) for l in range(2)]
    eps_d = din("eps", [P, 1], f32)
    wc_d = din("wc", [DOUT, NCLS], f16)
    bc_d = din("bc", [NCLS, 1], f32)
    out_d = nc.dram_tensor("out", [NCLS, NPAD], f32, kind="ExternalOutput")

    xg_own = [nc.dram_tensor(f"xg{l}_own", [NPC, D], f16) for l in range(2)]
    xg_full = [
        nc.dram_tensor(f"xg{l}_full", [N, D], f16, addr_space="Shared")
        for l in range(2)
    ]

    with tile.TileContext(nc) as tc, ExitStack() as ctx:
        cpool = ctx.enter_context(tc.tile_pool(name="const", bufs=1))
        xpool = ctx.enter_context(tc.tile_pool(name="x", bufs=1))
        idxpool = ctx.enter_context(tc.tile_pool(name="idx", bufs=3))
        msgApool = ctx.enter_context(tc.tile_pool(name="msgA", bufs=2))
        msgBpool = ctx.enter_context(tc.tile_pool(name="msgB", bufs=2))
        ohpool = ctx.enter_context(tc.tile_pool(name="oh", bufs=6))
        wkpool = ctx.enter_context(tc.tile_pool(name="wk", bufs=4))
        stpool = ctx.enter_context(tc.tile_pool(name="st", bufs=8))
        psA = ctx.enter_context(tc.tile_pool(name="psA", bufs=2, space="PSUM"))
        psC = ctx.enter_context(tc.tile_pool(name="psC", bufs=2, space="PSUM"))
        psT = ctx.enter_context(tc.tile_pool(name="psT", bufs=3, space="PSUM"))

        def load(dram, shape, dt):
            t = cpool.tile(shape, dt, name=f"c_{dram.name}")
            nc.sync.dma_start(out=t[:], in_=dram.ap())
            return t

        iota_t = load(iota_d, [P, P], f16)
        ident_t = load(ident_d, [P, P], f32)
        relA_t = load(relA_d, [P, TL], f32)
        relB_t = load(relB_d, [P, TH], f32)
        invA_t = load(invA_d, [P, TL], f32)
        invB_t = load(invB_d, [P, TH], f32)
        wl_t = [load(wl_d[l], [D, D if l < 2 else DOUT], f16) for l in range(3)]
        wr_t = [load(wr_d[l], [D, D if l < 2 else DOUT], f16) for l in range(3)]
        bl_t = [load(bl_d[l], [D if l < 2 else DOUT, 1], f32) for l in range(3)]
        g_t = [load(g_d[l], [D, 1], f32) for l in range(2)]
        b_t = [load(b_d[l], [D, 1], f32) for l in range(2)]
        eps_t = load(eps_d, [P, 1], f32)
        wc_t = load(wc_d, [DOUT, NCLS], f16)
        bc_t = load(bc_d, [NCLS, 1], f32)

        xfm32 = [xpool.tile([P, NPAD], f32, tag=f"xfm32_{i}", name=f"xfm32_{i}")
                 for i in range(2)]
        xfm16 = [xpool.tile([P, NPAD], f16, tag=f"xfm16_{i}", name=f"xfm16_{i}")
                 for i in range(2)]
        nc.sync.dma_start(out=xfm32[0][:], in_=xfm32_d.ap())
        nc.sync.dma_start(out=xfm16[0][:], in_=xfm16_d.ap())
        x3fm = xpool.tile([DOUT, NPAD], f32, tag="x3fm")
        normfm = xpool.tile([DOUT, NPAD], f16, tag="normfm")

        nchunkA = (TL + G - 1) // G
        nchunkB = (TH + G - 1) // G

        for l in range(3):
            dout = D if l < 2 else DOUT
            cur32, cur16 = xfm32[l % 2], xfm16[l % 2]
            nxt32, nxt16 = xfm32[(l + 1) % 2], xfm16[(l + 1) % 2]
            if l == 0:
                src_lo = xf16_d.ap()[:HALF, :]
                src_hi = xf16_d.ap()[HALF:, :]
            else:
                src_lo = xg_full[l - 1].ap()[:HALF, :]
                src_hi = xg_full[l - 1].ap()[HALF:, :]

            # emit gather chunks lazily; Tile pool backpressure pipelines them
            msgs = {"A": {}, "B": {}}
            issued = {"A": -1, "B": -1}

            def emit_chunk(stream, ci, l=l, src_lo=src_lo, src_hi=src_hi,
                           msgs=msgs):
                Tn = TL if stream == "A" else TH
                idxd = idxA_d if stream == "A" else idxB_d
                mpool = msgApool if stream == "A" else msgBpool
                src = src_lo if stream == "A" else src_hi
                c0 = ci * G
                cn = min(G, Tn - c0)
                nidx = cn * P
                it = idxpool.tile([P, G * 8], i16, tag="idx")
                nc.sync.dma_start(
                    out=it[:, : cn * 8], in_=idxd.ap()[:, c0 * 8 : c0 * 8 + cn * 8]
                )
                mt = mpool.tile([P, G * P], f16, tag=f"msg{stream}")
                nc.gpsimd.dma_gather(
                    mt[:, : cn * P].rearrange("p (t d) -> p t d", d=P),
                    src,
                    it[:, : cn * 8],
                    nidx,
                    nidx,
                    P,
                    single_packet=False,
                )
                msgs[stream][ci] = mt

            for w in range(W):
                nA, nB = int(ntiles[w, 0]), int(ntiles[w, 1])
                subs = [("A", int(startA[w]) + i) for i in range(nA)] + [
                    ("B", int(startB[w]) + i) for i in range(nB)
                ]
                for stream, pos in subs:
                    while issued[stream] < pos // G:
                        issued[stream] += 1
                        emit_chunk(stream, issued[stream])

                ps = psA.tile([P, P], f32, space="PSUM", tag="agg")
                for si, (stream, pos) in enumerate(subs):
                    rel_t = relA_t if stream == "A" else relB_t
                    inv_t = invA_t if stream == "A" else invB_t
                    mt = msgs[stream][pos // G]
                    t = pos % G
                    oh = ohpool.tile([P, P], f16, tag="oh")
                    nc.vector.tensor_scalar(
                        out=oh[:],
                        in0=iota_t[:],
                        scalar1=rel_t[:, pos : pos + 1],
                        scalar2=inv_t[:, pos : pos + 1],
                        op0=mybir.AluOpType.is_equal,
                        op1=mybir.AluOpType.mult,
                    )
                    nc.tensor.matmul(
                        out=ps[:],
                        lhsT=mt[:, t * P : (t + 1) * P],
                        rhs=oh[:],
                        start=(si == 0),
                        stop=(si == len(subs) - 1),
                    )
                agg16 = wkpool.tile([P, P], f16, tag="agg16")
                nc.scalar.copy(agg16[:], ps[:])

                hps = psC.tile([dout, P], f32, space="PSUM", tag="h")
                nc.tensor.matmul(out=hps[:], lhsT=wl_t[l][:], rhs=agg16[:],
                                 start=True, stop=False)
                nc.tensor.matmul(out=hps[:], lhsT=wr_t[l][:],
                                 rhs=cur16[:, w * P : (w + 1) * P],
                                 start=False, stop=True)

                cols = slice(w * P, (w + 1) * P)
                if l < 2:
                    hfm = wkpool.tile([P, P], f32, tag="hfm")
                    nc.scalar.activation(hfm[:], hps[:],
                                         mybir.ActivationFunctionType.Identity,
                                         bias=bl_t[l][:, :1])
                    tp1 = psT.tile([P, P], f32, space="PSUM", tag="tp")
                    nc.tensor.transpose(tp1[:], hfm[:], ident_t[:])
                    hnm = wkpool.tile([P, P], f32, tag="hnm")
                    nc.scalar.copy(hnm[:], tp1[:])
                    s_ = stpool.tile([P, 1], f32, tag="sum")
                    nc.vector.reduce_sum(s_[:], hnm[:], axis=mybir.AxisListType.X)
                    nmu = stpool.tile([P, 1], f32, tag="nmu")
                    nc.scalar.mul(nmu[:], s_[:], -1.0 / D)
                    xc = wkpool.tile([P, P], f32, tag="xc")
                    nc.scalar.activation(xc[:], hnm[:],
                                         mybir.ActivationFunctionType.Identity,
                                         bias=nmu[:, :1])
                    sq = wkpool.tile([P, P], f32, tag="sq")
                    ss = stpool.tile([P, 1], f32, tag="ss")
                    nc.scalar.activation(sq[:], xc[:],
                                         mybir.ActivationFunctionType.Square,
                                         accum_out=ss[:, :1])
                    sd = stpool.tile([P, 1], f32, tag="sd")
                    nc.scalar.activation(sd[:], ss[:],
                                         mybir.ActivationFunctionType.Sqrt,
                                         scale=1.0 / D, bias=eps_t[:, :1])
                    rs = stpool.tile([P, 1], f32, tag="rs")
                    nc.vector.reciprocal(rs[:], sd[:])
                    nrm = wkpool.tile([P, P], f32, tag="nrm")
                    nc.vector.tensor_scalar_mul(nrm[:], xc[:], rs[:, :1])
                    tp2 = psT.tile([P, P], f32, space="PSUM", tag="tp")
                    nc.tensor.transpose(tp2[:], nrm[:], ident_t[:])
                    gel = wkpool.tile([P, P], f32, tag="gel")
                    nc.scalar.activation(gel[:], tp2[:],
                                         mybir.ActivationFunctionType.Gelu,
                                         bias=b_t[l][:, :1], scale=g_t[l][:, :1])
                    nc.vector.tensor_add(nxt32[:, cols], gel[:], cur32[:, cols])
                    nc.any.tensor_copy(nxt16[:, cols], nxt32[:, cols])
                    tp3 = psT.tile([P, P], f32, space="PSUM", tag="tp")
                    nc.tensor.transpose(tp3[:], nxt32[:, cols], ident_t[:])
                    xnm = wkpool.tile([P, P], f16, tag="xnm")
                    nc.scalar.copy(xnm[:], tp3[:])
                    rows = min(P, NPC - w * P)
                    nc.sync.dma_start(
                        out=xg_own[l].ap()[w * P : w * P + rows, :],
                        in_=xnm[:rows, :],
                    )
                else:
                    nc.scalar.activation(x3fm[:, cols], hps[:],
                                         mybir.ActivationFunctionType.Gelu,
                                         bias=bl_t[l][:, :1])

            if l < 2:
                nc.gpsimd.collective_compute(
                    "AllGather",
                    mybir.AluOpType.bypass,
                    replica_groups=[list(range(NCORES))],
                    ins=[xg_own[l].ap()],
                    outs=[xg_full[l].ap()],
                )

        # classifier: LN (affine folded into wc) then linear
        for w in range(W):
            cols = slice(w * P, (w + 1) * P)
            tp1 = psT.tile([P, DOUT], f32, space="PSUM", tag="tp")
            nc.tensor.transpose(tp1[:], x3fm[:, cols], ident_t[:DOUT, :DOUT])
            ynm = wkpool.tile([P, DOUT], f32, tag="ynm")
            nc.scalar.copy(ynm[:], tp1[:])
            s_ = stpool.tile([P, 1], f32, tag="sum")
            nc.vector.reduce_sum(s_[:], ynm[:], axis=mybir.AxisListType.X)
            nmu = stpool.tile([P, 1], f32, tag="nmu")
            nc.scalar.mul(nmu[:], s_[:], -1.0 / DOUT)
            xc = wkpool.tile([P, DOUT], f32, tag="xc")
            nc.scalar.activation(xc[:], ynm[:],
                                 mybir.ActivationFunctionType.Identity,
                                 bias=nmu[:, :1])
            sq = wkpool.tile([P, DOUT], f32, tag="sq")
            ss = stpool.tile([P, 1], f32, tag="ss")
            nc.scalar.activation(sq[:], xc[:],
                                 mybir.ActivationFunctionType.Square,
                                 accum_out=ss[:, :1])
            sd = stpool.tile([P, 1], f32, tag="sd")
            nc.scalar.activation(sd[:], ss[:],
                                 mybir.ActivationFunctionType.Sqrt,
                                 scale=1.0 / DOUT, bias=eps_t[:, :1])
            rs = stpool.tile([P, 1], f32, tag="rs")
            nc.vector.reciprocal(rs[:], sd[:])
            nrm = wkpool.tile([P, DOUT], f32, tag="nrm")
            nc.vector.tensor_scalar_mul(nrm[:], xc[:], rs[:, :1])
            tp2 = psT.tile([DOUT, P], f32, space="PSUM", tag="tp")
            nc.tensor.transpose(tp2[:], nrm[:], ident_t[:])
            nc.scalar.copy(normfm[:, cols], tp2[:])

        NCHUNK = 512
        for c0 in range(0, NPAD, NCHUNK):
            cn = min(NCHUNK, NPAD - c0)
            ops = psC.tile([NCLS, NCHUNK], f32, space="PSUM", tag="h")
            nc.tensor.matmul(out=ops[:, :cn], lhsT=wc_t[:],
                             rhs=normfm[:, c0 : c0 + cn], start=True, stop=True)
            osb = wkpool.tile([NCLS, NCHUNK], f32, tag="osb")
            nc.scalar.activation(osb[:, :cn], ops[:, :cn],
                                 mybir.ActivationFunctionType.Identity,
                                 bias=bc_t[:, :1])
            nc.sync.dma_start(out=out_d.ap()[:, c0 : c0 + cn], in_=osb[:, :cn])

    nc.compile()
    return nc


def _prep_inputs(x, sched, weights):
    """Build per-core input maps."""
    xf16 = x.astype(np.float16)
    iota = np.broadcast_to(np.arange(P, dtype=np.float16), (P, P)).copy()
    ident = np.eye(P, dtype=np.float32)
    (Wl1, bl1, Wr1, g1, b1, Wl2, bl2, Wr2, g2, b2,
     Wl3, bl3, Wr3, gc, bc, Wc, bcls) = weights
    wcp = (gc[:, None].astype(np.float32) * Wc.astype(np.float32))
    bcp = bc.astype(np.float32) @ Wc.astype(np.float32) + bcls.astype(np.float32)
    common = {
        "xf16": xf16,
        "iota": iota,
        "ident": ident,
        "eps": np.full((P, 1), LN_EPS, np.float32),
        "wl0": Wl1.astype(np.float16), "wr0": Wr1.astype(np.float16),
        "wl1": Wl2.astype(np.float16), "wr1": Wr2.astype(np.float16),
        "wl2": Wl3.astype(np.float16), "wr2": Wr3.astype(np.float16),
        "bl0": bl1.reshape(-1, 1).astype(np.float32),
        "bl1": bl2.reshape(-1, 1).astype(np.float32),
        "bl2": bl3.reshape(-1, 1).astype(np.float32),
        "g0": g1.reshape(-1, 1).astype(np.float32),
        "b0": b1.reshape(-1, 1).astype(np.float32),
        "g1": g2.reshape(-1, 1).astype(np.float32),
        "b1": b2.reshape(-1, 1).astype(np.float32),
        "wc": wcp.astype(np.float16),
        "bc": bcp.reshape(-1, 1).astype(np.float32),
    }
    in_maps = []
    for c in range(NCORES):
        xc_ = x[c * NPC : (c + 1) * NPC].astype(np.float32)
        xfm = np.zeros((P, NPAD), np.float32)
        xfm[:, :NPC] = xc_.T
        m = dict(common)
        m.update(
            xfm32=xfm,
            xfm16=xfm.astype(np.float16),
            idxA=sched["idxA"][c],
            idxB=sched["idxB"][c],
            relA=sched["relA"][c],
            relB=sched["relB"][c],
            invA=sched["invA"][c],
            invB=sched["invB"][c],
        )
        in_maps.append(m)
    return in_maps




class _Runner:
    """Persistent PJRT runner: traces/compiles once, keeps inputs on device,
    supports steady-state timing of repeated executions."""

    def __init__(self, nc, in_maps):
        import jax
        from jax.sharding import Mesh, PartitionSpec
        try:
            from jax.experimental.shard_map import shard_map
        except ImportError:
            from jax.shard_map import shard_map
        from concourse import bass2jax, mybir as mb

        bass2jax.install_neuronx_cc_hook()
        self.jax = jax
        partition_name = (
            nc.partition_id_tensor.name if nc.partition_id_tensor else None
        )
        in_names, out_names, out_avals, zero_outs = [], [], [], []
        for alloc in nc.m.functions[0].allocations:
            if not isinstance(alloc, mb.MemoryLocationSet):
                continue
            name = alloc.memorylocations[0].name
            if alloc.kind == "ExternalInput":
                if name != partition_name:
                    in_names.append(name)
            elif alloc.kind == "ExternalOutput":
                out_names.append(name)
                shape = tuple(alloc.tensor_shape)
                dtype = mb.dt.np(alloc.dtype)
                out_avals.append(jax.core.ShapedArray(shape, dtype))
                zero_outs.append(np.zeros(shape, dtype))
        n_params = len(in_names)
        all_names = in_names + out_names
        if partition_name is not None:
            all_names.append(partition_name)

        def _body(*args):
            operands = list(args)
            if partition_name is not None:
                operands.append(bass2jax.partition_id_tensor())
            outs = bass2jax._bass_exec_p.bind(
                *operands,
                out_avals=tuple(out_avals),
                in_names=tuple(all_names),
                out_names=tuple(out_names),
                lowering_input_output_aliases=(),
                sim_require_finite=True,
                sim_require_nnan=True,
                nc=nc,
            )
            return tuple(outs)

        devices = jax.devices()[:NCORES]
        mesh = Mesh(np.asarray(devices), ("core",))
        n_outs = len(out_avals)
        self.fn = jax.jit(
            shard_map(
                _body,
                mesh=mesh,
                in_specs=(PartitionSpec("core"),) * (n_params + n_outs),
                out_specs=(PartitionSpec("core"),) * n_outs,
                check_rep=False,
            ),
            keep_unused=True,
        )
        self.out_names = out_names
        self.out_avals = out_avals
        concat_in = [
            np.concatenate([np.asarray(in_maps[c][nm]) for c in range(NCORES)])
            for nm in in_names
        ]
        concat_zeros = [
            np.concatenate([z] * NCORES, axis=0) for z in zero_outs
        ]
        self.dev_args = [jax.device_put(a) for a in concat_in + concat_zeros]
        self.update_idx = {nm: i for i, nm in enumerate(in_names)}
        self.in_names = in_names

    def refresh(self, in_maps):
        for nm in self.in_names:
            arr = np.concatenate(
                [np.asarray(in_maps[c][nm]) for c in range(NCORES)]
            )
            self.dev_args[self.update_idx[nm]] = self.jax.device_put(arr)

    def update_input(self, name, per_core_arrays):
        arr = np.concatenate([np.asarray(a) for a in per_core_arrays])
        self.dev_args[self.update_idx[name]] = self.jax.device_put(arr)

    def run(self):
        outs = self.fn(*self.dev_args)
        self.jax.block_until_ready(outs)
        return [
            {
                nm: np.asarray(outs[i]).reshape(NCORES, *self.out_avals[i].shape)[c]
                for i, nm in enumerate(self.out_names)
            }
            for c in range(NCORES)
        ]

    def time(self, reps=20, warmup=2):
        import time as _time
        for _ in range(warmup):
            self.jax.block_until_ready(self.fn(*self.dev_args))
        t0 = _time.time()
        outs = None
        for _ in range(reps):
            outs = self.fn(*self.dev_args)
        self.jax.block_until_ready(outs)
        return (_time.time() - t0) / reps


def kernel(x, edge_index, Wl1, bl1, Wr1, g1, b1, Wl2, bl2, Wr2, g2, b2,
           Wl3, bl3, Wr3, gc, bc, Wc, bcls):
    x = np.asarray(x)
    edge_index = np.asarray(edge_index)
    runner = get_runner(x, edge_index, Wl1, bl1, Wr1, g1, b1, Wl2, bl2, Wr2,
                        g2, b2, Wl3, bl3, Wr3, gc, bc, Wc, bcls)
    results = runner.run()
    out = np.empty((N, NCLS), np.float32)
    for c in range(NCORES):
        out[c * NPC : (c + 1) * NPC] = results[c]["out"][:, :NPC].T
    return out


def get_runner(x, edge_index, Wl1, bl1, Wr1, g1, b1, Wl2, bl2, Wr2, g2, b2,
               Wl3, bl3, Wr3, gc, bc, Wc, bcls):
    x = np.asarray(x)
    edge_index = np.asarray(edge_index)
    sched = _schedule(edge_index)
    key = (sched["TL"], sched["TH"], tuple(sched["ntiles"].ravel().tolist()))
    if key not in _cache:
        _cache[key] = _build(sched)
    nc = _cache[key]
    weights = (Wl1, bl1, Wr1, g1, b1, Wl2, bl2, Wr2, g2, b2,
               Wl3, bl3, Wr3, gc, bc, Wc, bcls)
    in_maps = _prep_inputs(x, sched, [np.asarray(w) for w in weights])
    rkey = ("runner", key)
    if rkey not in _cache:
        _cache[rkey] = _Runner(nc, in_maps)
    else:
        _cache[rkey].refresh(in_maps)
    return _cache[rkey]
